# revision 27
# baseline (speedup 1.0000x reference)
"""Trainium2 Bass kernel for CDAttnBlock (v3 — DMA transposes + fast lead-in).

Reference computation (per batch element b, all in fp32):
    q,k,v   = split(x  @ Wqkv)   heads=12, d=64
    q2,k2,v2= split(x2 @ Wqkv)
    o1 = attn(q, k,  v);  o2 = attn(q2, k2, v2);  o3 = attn(q, k2, v2)
    y_i = merge(o_i) @ Wout + bout

Sharding: pure data-parallel over batch (B=8) across 8 NeuronCores.

v3 design (vs v2 at ~526us):
  - All x/x2 transposes move off the PE (they ran as fp32 PE transposes:
    ~420ns MM + ~225ns exposed LDW x96 = ~60us of PE time) onto the DMA
    xbar: per s-tile, one f16 cast + ONE transpose DMA with a 3D output
    AP covering all 6 kt-column blocks of xT.
  - Lead-in: strided priming DMAs pull only the pair-0 q, pair-0/1 k and
    head-0/1 v columns of Wqkv so the first score matmul + exp fire at
    ~18us (vs ~36us); the rest of Wqkv streams in pair-aligned 256-col
    block DMAs through a small rotating stage and is cast on DVE (casts
    were ScalarE ACTIVATE COPYs = 17us of the exp engine). 16 junk f16
    matmuls warm the PE HAM clock-gate before real work arrives.
  - Work-item deadlines are set so no emitted matmul waits on HBM data
    (an in-order engine queue head that blocks stalls everything behind
    it).
  - The three den4 tile sets are one buffer set: each later attention
    re-initializes them right after the previous attention's normalize
    chain has consumed them (recip -> memset, program-ordered).
  - Tail: attn2's last den group (heads 8-11) normalizes its qh0 half
    mid-attention (site 88); only the qh1 half + y2 projection remain
    after the exp stream ends.
  - PSUM: scores 2x[128,1024] (4 banks) + ov 2x[65,512] (2) + aux 2x
    [128,512] (2) = 8 banks.
"""

import numpy as np

import concourse.bass as bass
import concourse.tile as tile
from concourse import bacc, mybir
from concourse.bass_utils import run_bass_kernel_spmd

F32 = mybir.dt.float32
F16 = mybir.dt.float16
AF = mybir.ActivationFunctionType

HIDDEN = 768
HEADS = 12
D = 64
S = 1024
B = 8
SCALE = D ** -0.5
NPAIR = HEADS // 2          # 6 head pairs
KT = HIDDEN // 128          # 6 k-tiles over hidden
ST = S // 128               # 8 s-tiles
VW = D + 1                  # 65: v columns + ones column


class Ctx:
    """Shared handles for the kernel builder."""


# ---------------------------------------------------------------------------
# aux thunk builders (each returns a list of closures; every closure is a
# small burst of engine work suitable for pumping into exp-stream gaps)
# ---------------------------------------------------------------------------

def th_xt_tdma(c, xT, st, xs3, st_in_stage):
    """One s-tile of x -> xT via DMA xbar transpose: one f16 cast, then a
    single transpose DMA whose 3D output AP covers all 6 kt-blocks."""
    nc = c.nc
    out3 = xT.rearrange("p (h s) -> p h s", s=S)

    def f():
        x16 = c.xnat.tile([128, HIDDEN], F16, name="x16", tag="x16")
        nc.vector.tensor_copy(x16[:], xs3[:, st_in_stage, :])
        nc.sync.dma_start(out3[:, :, st * 128:(st + 1) * 128], x16[:],
                          transpose=True)
    return [f]


def th_v(c, xT, v_st, st, lo, hi, ha, hb):
    """v columns [lo,hi) of Wqkv's v-block = heads [ha,hb) of s-tile st."""
    nc = c.nc

    def f():
        vp = c.auxp.tile([128, hi - lo], F32, name="vp", tag=c.aux_tag())
        for kt in range(KT):
            nc.tensor.matmul(
                vp[:], xT[:, kt * S + st * 128:kt * S + (st + 1) * 128],
                c.wq16v[:, kt, 2 * HIDDEN + lo:2 * HIDDEN + hi],
                start=(kt == 0), stop=(kt == KT - 1))
        vs3 = v_st[st].rearrange("p (h w) -> p h w", w=VW)
        nc.vector.tensor_copy(
            vs3[:, ha:hb, 0:D], vp.rearrange("p (h w) -> p h w", w=D))
    return [f]


def th_qk(c, xT, p, base, dst):
    """qT or kT for one pair: two halves x two 3-kt sub-thunks each
    (small bursts keep the exp stream fed)."""
    nc = c.nc

    def half(hf):
        hold = {}

        def fa():
            hold["pp"] = c.auxp.tile([128, 512], F32, name="qkp",
                                     tag=c.aux_tag())
            lo = hf * 512
            for kt in range(3):
                nc.tensor.matmul(
                    hold["pp"][:],
                    c.wq16v[:, kt, base + p * 128:base + (p + 1) * 128],
                    xT[:, kt * S + lo:kt * S + lo + 512],
                    start=(kt == 0), stop=False)

        def fb():
            lo = hf * 512
            for kt in range(3, KT):
                nc.tensor.matmul(
                    hold["pp"][:],
                    c.wq16v[:, kt, base + p * 128:base + (p + 1) * 128],
                    xT[:, kt * S + lo:kt * S + lo + 512],
                    start=False, stop=(kt == KT - 1))
            nc.vector.tensor_copy(dst[p][:, lo:lo + 512], hold["pp"][:])
        return [fa, fb]
    return half(0) + half(1)


def th_proj(c, oT, y_dram, st, psum_cycle=None):
    """y[st] = oT.T @ Wout + bias -> DRAM; two half-thunks.
    psum_cycle: optional callable yielding (pool, tag) — used in the
    tail to rotate over 4 free PSUM banks instead of the 2 aux banks."""
    nc = c.nc
    hold = {}

    def half(h):
        def f():
            lo, hi = (0, 512) if h == 0 else (512, 768)
            if psum_cycle is None:
                yp = c.auxp.tile([128, hi - lo], F32, name="yp",
                                 tag=c.aux_tag())
            else:
                pool, tag = psum_cycle()
                yp = pool.tile([128, hi - lo], F32, name="yp", tag=tag)
            for ct in range(KT):
                nc.tensor.matmul(
                    yp[:], oT[ct][:, st * 128:(st + 1) * 128],
                    c.wout16v[:, ct, lo:hi],
                    start=(ct == 0), stop=(ct == KT - 1))
            if h == 0:
                hold["yt"] = c.ysb.tile([128, HIDDEN], F32, name="yt",
                                        tag="yt")
            yt = hold["yt"]
            nc.vector.tensor_add(yt[:, lo:hi], yp[:], c.bias_sb[:, lo:hi])
            if h == 1:
                nc.sync.dma_start(y_dram[st * 128:(st + 1) * 128, :], yt[:])
        return f
    return [half(0), half(1)]


def norm_group(c, oT, den4s, g, qh):
    """Normalize chain for one den group (4 heads) over query half qh:
    upcast + fast-approx reciprocal + downcast, then per-head gpsimd
    bcast + in-place f16 multiply on oT. Tags are shared across calls
    (sequential reuse is serialized by the tile dep tracker)."""
    nc = c.nc
    hold = {}
    qsl = slice(qh * 512, (qh + 1) * 512)
    qn = 512

    def t_recip():
        def f():
            df = c.dnp.tile([128, qn], F32, name="df", tag="dfh", bufs=1)
            nc.vector.tensor_copy(df[:], den4s[g][:, qsl])
            nc.vector.reciprocal_approx_fast(df[:], df[:])
            r16 = c.dnp.tile([128, qn], F16, name="r16",
                             tag=f"r16g{g}", bufs=1)
            nc.vector.tensor_copy(r16[:], df[:])
            hold["r"] = r16
        return f

    def t_head(j):
        def f():
            h = 4 * g + j
            p, hh = h // 2, h % 2
            hp = slice(hh * D, (hh + 1) * D)
            # partition_broadcast only supports src/dst partition 0, so
            # stage the recip row down to partition 0, then broadcast to
            # all 128 and multiply against the matching half (tensor ops
            # need equal input base partitions).
            rrow = c.bcsp.tile([1, qn], F16, name="rrow", tag="rrow",
                               bufs=1)
            nc.vector.tensor_copy(rrow[:], hold["r"][32 * j:32 * j + 1, :])
            bcs = c.bcsp.tile([128, qn], F16, name="bcs", tag="bcs",
                              bufs=2)
            nc.gpsimd.partition_broadcast(bcs[:], rrow[:])
            nc.vector.tensor_mul(oT[p][hp, qsl], oT[p][hp, qsl],
                                 bcs[hp, :])
        return f

    return [t_recip()] + [t_head(j) for j in range(4)]


# ---------------------------------------------------------------------------
# the attention pipeline
# ---------------------------------------------------------------------------

def attention(c, qT, kT, v_st, oT, den4s, work, first_inline=None,
              at_site=None):
    """One attention (12 heads as 6 row-tiled pairs x 2 q-halves x 8
    key-tiles).

    `work` = list of (earliest, latest, thunk) in step units (site*ST+kt,
    0..96): thunks are pumped into the exp-stream gaps at an even pace
    but never before `earliest` (so an emitted op never camps on an
    engine queue waiting for HBM data) and always before the sweep whose
    site index reaches `latest` (program order on each engine queue is
    the dependency order -- a consumer emitted before its producer reads
    garbage).
    `first_inline`: 8 thunk-lists run inside the first sweep, list[kt]
    right after exp(kt) and before av(kt) (used to produce v and the
    second q/k halves just in time for the very first attention)."""
    nc = c.nc
    pending = sorted(work, key=lambda t: t[1])
    state = {"credit": 0.0}
    rate = max(0.001, len(pending) / 96.0)

    def emit_one(now):
        for idx in range(len(pending)):
            if pending[idx][0] <= now:
                _, _, f = pending.pop(idx)
                f()
                return True
        return False

    def pump(now):
        state["credit"] = min(state["credit"] + rate, 6.0)
        while state["credit"] >= 1.0 and pending:
            if not emit_one(now):
                break
            state["credit"] -= 1.0

    def force(site):
        i = 0
        while i < len(pending):
            if pending[i][1] <= site:
                _, _, f = pending.pop(i)
                f()
            else:
                i += 1

    for pair in range(NPAIR):
        for qh in range(2):
            site = (pair * 2 + qh) * ST
            force(site)
            if at_site is not None:
                for f in at_site.pop(site, []):
                    f()
            qsl = slice(qh * 512, (qh + 1) * 512)
            ovA = c.ovps.tile([VW, 512], F32, name="ovA", tag="ovA")
            ovB = c.ovps.tile([VW, 512], F32, name="ovB", tag="ovB")
            for kt in range(ST):
                sp = c.sps.tile([128, S], F32, name="sp", tag="sp")
                ksl = slice(kt * 128, (kt + 1) * 128)
                nc.tensor.matmul(sp[:, 0:512], kT[pair][0:D, ksl],
                                 qT[pair][0:D, qsl], start=True, stop=True)
                nc.tensor.matmul(sp[:, 512:1024], kT[pair][D:128, ksl],
                                 qT[pair][D:128, qsl], start=True, stop=True)
                ex = c.exps.tile([128, S], F16, name="ex", tag="ex")
                nc.scalar.activation(ex[:], sp[:], AF.Exp,
                                     bias=c.zbias[:], scale=SCALE)
                if first_inline is not None and pair == 0 and qh == 0:
                    for f in first_inline[kt]:
                        f()
                vs3 = v_st[kt].rearrange("q (h w) -> q h w", w=VW)
                nc.tensor.matmul(ovA[:], vs3[:, 2 * pair, :], ex[:, 0:512],
                                 start=(kt == 0), stop=(kt == ST - 1))
                nc.tensor.matmul(ovB[:], vs3[:, 2 * pair + 1, :],
                                 ex[:, 512:1024],
                                 start=(kt == 0), stop=(kt == ST - 1))
                pump(site + kt)
            # sweep tail: evacuate o (f16, pre-normalize) + denominators
            # (den row h goes to partition 32*(h%4) of group tile h//4 —
            # DVE moves must keep partition start congruent mod 32)
            hA, hB = 2 * pair, 2 * pair + 1
            nc.vector.tensor_copy(oT[pair][0:D, qsl], ovA[0:D, :])
            nc.vector.tensor_copy(
                den4s[hA // 4][32 * (hA % 4):32 * (hA % 4) + 1, qsl],
                ovA[D:VW, :])
            nc.vector.tensor_copy(oT[pair][D:128, qsl], ovB[0:D, :])
            nc.vector.tensor_copy(
                den4s[hB // 4][32 * (hB % 4):32 * (hB % 4) + 1, qsl],
                ovB[D:VW, :])
    # drain leftovers
    for _, _, f in pending:
        f()


def build_kernel(ctx, tc, x, x2, wq, wo, bo, y1, y2, y3):
    nc = tc.nc
    c = Ctx()
    c.nc = nc
    c._aux_flip = [0]

    def aux_tag():
        c._aux_flip[0] ^= 1
        return ("auxA", "auxB")[c._aux_flip[0]]
    c.aux_tag = aux_tag

    # ---------------- constants ---------------------------------------
    const = ctx.enter_context(tc.tile_pool(name="const", bufs=1))
    c.zbias = const.tile([128, 1], F32, name="zbias")
    nc.vector.memset(c.zbias[:], 0.0)
    c.bias_sb = const.tile([128, HIDDEN], F32, name="bias_sb")
    warm = const.tile([128, 256], F16, name="warm")
    nc.vector.memset(warm[:], 0.0)

    # ---------------- persistent pools --------------------------------
    woutp = ctx.enter_context(tc.tile_pool(name="woutp", bufs=1))
    wout16 = woutp.tile([128, KT * HIDDEN], F16, name="wout16")
    c.wout16v = wout16.rearrange("p (kt ch) -> p kt ch", ch=HIDDEN)
    qxp = ctx.enter_context(tc.tile_pool(name="qxp", bufs=1))
    qT_x = [qxp.tile([128, S], F16, name=f"qTx{i}", tag=f"qTx{i}")
            for i in range(NPAIR)]
    kvx2p = ctx.enter_context(tc.tile_pool(name="kvx2p", bufs=1))
    kT_x2 = [kvx2p.tile([128, S], F16, name=f"kTx2{i}", tag=f"kTx2{i}")
             for i in range(NPAIR)]
    v_x2 = [kvx2p.tile([128, HEADS * VW], F16, name=f"vx2{i}",
                       tag=f"vx2{i}") for i in range(ST)]
    otp = ctx.enter_context(tc.tile_pool(name="otp", bufs=1))
    oT1 = [otp.tile([128, S], F16, name=f"oTa{i}", tag=f"oTa{i}")
           for i in range(NPAIR)]

    # pools released mid-build (allocated after the persistent ones)
    x2tp = tc.alloc_tile_pool(name="x2tp", bufs=1)
    x2T = x2tp.tile([128, KT * S], F16, name="x2T")
    wqp = tc.alloc_tile_pool(name="wqp", bufs=1)
    wq16 = wqp.tile([128, KT * 3 * HIDDEN], F16, name="wq16")
    c.wq16v = wq16.rearrange("p (kt ch) -> p kt ch", ch=3 * HIDDEN)
    kvxp = tc.alloc_tile_pool(name="kvxp", bufs=1)
    kT_x = [kvxp.tile([128, S], F16, name=f"kTx{i}", tag=f"kTx{i}")
            for i in range(NPAIR)]
    v_x = [kvxp.tile([128, HEADS * VW], F16, name=f"vx{i}", tag=f"vx{i}")
           for i in range(ST)]
    xtp = tc.alloc_tile_pool(name="xtp", bufs=1)
    xT = xtp.tile([128, KT * S], F16, name="xT")
    xsp = tc.alloc_tile_pool(name="xsp", bufs=1)
    xsA = xsp.tile([128, 4 * HIDDEN], F32, name="xsA", tag="xsA")
    xsA3 = xsA.rearrange("p (st h) -> p st h", h=HIDDEN)
    wsp = tc.alloc_tile_pool(name="wsp", bufs=1)
    # priming stages: pair-0 q, pair-0 k, head-0/1 v columns
    wqa = wsp.tile([128, KT * 128], F32, name="wqa", tag="wqa")
    wqb = wsp.tile([128, KT * 128], F32, name="wqb", tag="wqb")
    wqc = wsp.tile([128, KT * 128], F32, name="wqc", tag="wqc")
    # two rotating 256-col block stages for the rest of Wqkv
    wblkA = wsp.tile([128, KT * 256], F32, name="wblkA", tag="wblkA")
    wblkB = wsp.tile([128, KT * 256], F32, name="wblkB", tag="wblkB")
    wblkA3 = wblkA.rearrange("p (kt ch) -> p kt ch", ch=256)
    wblkB3 = wblkB.rearrange("p (kt ch) -> p kt ch", ch=256)

    # ---------------- working pools (right side) ----------------------
    c.xnat = tc.alloc_tile_pool(name="xnat", bufs=2, side="right")
    c.exps = tc.alloc_tile_pool(name="exps", bufs=3, side="right")
    c.dnp = tc.alloc_tile_pool(name="dnp", bufs=2, side="right")
    c.bcsp = tc.alloc_tile_pool(name="bcsp", bufs=1, side="right")

    # ---------------- PSUM pools --------------------------------------
    c.sps = tc.alloc_tile_pool(name="sps", bufs=2, space="PSUM")
    c.ovps = tc.alloc_tile_pool(name="ovps", bufs=1, space="PSUM")
    c.auxp = tc.alloc_tile_pool(name="auxp", bufs=1, space="PSUM")

    # ---------------- lead-in -----------------------------------------
    # DMA issue order is the HBM schedule: x half0, q/k/v primes, x half1
    # (needed by sweep-0 kt=4), v-block A, x2 half0, bias; the remaining
    # Wqkv blocks are work1 items paced so no emitted matmul ever waits.
    xr = x.rearrange("(st p) h -> p st h", p=128)
    x2r = x2.rearrange("(st p) h -> p st h", p=128)
    wr = wq.rearrange("(kt p) ch -> p kt ch", p=128)
    nc.sync.dma_start(xsA3[:, :, :], xr[:, 0:4, :])
    nc.sync.dma_start(wqa.rearrange("p (kt cc) -> p kt cc", cc=128),
                      wr[:, :, 0:128])
    nc.sync.dma_start(wqb.rearrange("p (kt cc) -> p kt cc", cc=128),
                      wr[:, :, HIDDEN:HIDDEN + 128])
    nc.sync.dma_start(wqc.rearrange("p (kt cc) -> p kt cc", cc=128),
                      wr[:, :, 2 * HIDDEN:2 * HIDDEN + 128])

    # PE warm-up: junk f16 matmuls pay the HAM cold penalty before any
    # real matmul arrives (~3.5us of activity from t~7us).
    for i in range(32):
        t = c.sps.tile([128, S], F32, name="sp", tag="sp")
        nc.tensor.matmul(t[:, 0:256], warm[:, 0:128], warm[:],
                         start=True, stop=True)

    # x half-0 casts + transpose DMAs, then reuse the stage for half-1
    for st in range(4):
        th_xt_tdma(c, xT, st, xsA3, st)[0]()
    nc.sync.dma_start(xsA3[:, :, :], xr[:, 4:8, :])
    for base, wt, cc in ((0, wqa, 128), (HIDDEN, wqb, 128),
                         (2 * HIDDEN, wqc, 128)):
        nc.vector.tensor_copy(
            c.wq16v[:, :, base:base + cc],
            wt.rearrange("p (kt cc) -> p kt cc", cc=cc))
    for st in range(4, 8):
        th_xt_tdma(c, xT, st, xsA3, st - 4)[0]()
    # v-block A (heads 2-5) right behind x half-1 on the HBM queue
    nc.sync.dma_start(wblkA3[:, :, :],
                      wr[:, :, 2 * HIDDEN + 128:2 * HIDDEN + 384])
    bo_bcast = bass.AP(tensor=bo.tensor, offset=bo.offset,
                       ap=[[0, 128]] + list(bo.ap))
    nc.sync.dma_start(c.bias_sb[:], bo_bcast)

    # ones columns of both v tensors (read by every av matmul)
    for vset in (v_x, v_x2):
        for st in range(ST):
            vs3 = vset[st].rearrange("p (h w) -> p h w", w=VW)
            nc.vector.memset(vs3[:, :, D:VW], 1.0)

    # prime pair-0 k and q, half 0 (the rest is inlined into sweep 0)
    k0 = th_qk(c, xT, 0, HIDDEN, kT_x)
    q0 = th_qk(c, xT, 0, 0, qT_x)
    for f in (k0[0], k0[1], q0[0], q0[1]):
        f()

    c.dnp_den = [c.dnp.tile([128, S], F16, name="den4", tag=f"den4{g}",
                            bufs=1) for g in range(3)]
    den = c.dnp_den
    for t in den:
        nc.vector.memset(t[:], 1.0)

    # ---------------- attn1 = attn(q, k, v) ---------------------------
    # first_inline[kt]: v heads 0/1 for s-tile kt (just in time for av)
    # plus the deferred pair-0 k/q half-1 sub-thunks.
    inline = [[th_v(c, xT, v_x, st, 0, 128, 0, 2)[0]] for st in range(ST)]
    inline[0].append(k0[2])
    inline[1].append(k0[3])
    inline[2].append(q0[2])
    inline[3].append(q0[3])

    def th_cast(kt, lo, hi, src):
        def f():
            nc.vector.tensor_copy(c.wq16v[:, kt, lo:hi], src)
        return f

    def th_dma(out_ap, in_ap):
        def f():
            nc.sync.dma_start(out_ap, in_ap)
        return f

    def blk(work, stage3, e_dma, l_dma, e_cast, l_cast, lo, width):
        """Stream Wqkv cols [lo, lo+width) through a rotating stage.
        DMA issue is cheap (sync queue) and pulled early; casts wait
        until the data can actually be resident. l_dma must respect the
        stage chain order (>= previous casts' l on the same stage)."""
        work.append((e_dma, l_dma, th_dma(stage3[:, :, 0:width],
                                          wr[:, :, lo:lo + width])))
        for kt in range(KT):
            work.append((e_cast + kt // 3, l_cast,
                         th_cast(kt, lo, lo + width,
                                 stage3[:, kt, 0:width])))

    H = HIDDEN
    END = NPAIR * 2 * ST + 1
    work1 = []
    # Wqkv block schedule. Two rotating stages; along each stage chain
    # (dma -> casts -> dma -> ...) both earliest and latest are strictly
    # non-decreasing, and every consumer has e/l >= its producers'.
    # HBM landings (serial queue): v-blkA ~24us (lead-in), k12 ~26,
    # q12 ~29, x2h0 ~33, v69 ~35, k34 ~37, q34 ~40, x2h1 ~46, k5 ~47,
    # v1011 ~48, q5 ~49. Wall(step) ~ 18 + step.
    for kt in range(KT):     # v-block A casts (heads 2-5; DMA above)
        work1.append((8 + kt // 3, 14, th_cast(
            kt, 2 * H + 128, 2 * H + 384, wblkA3[:, kt, :])))
    blk(work1, wblkB3, 1, 6, 10, 15, H + 128, 256)        # k pairs 1-2
    blk(work1, wblkA3, 9, 14, 13, 16, 128, 256)           # q pairs 1-2
    work1.append((10, 14, th_dma(xsA3[:, :, :], x2r[:, 0:4, :])))
    blk(work1, wblkB3, 11, 16, 19, 38, 2 * H + 384, 256)  # v heads 6-9
    blk(work1, wblkA3, 14, 17, 21, 44, H + 384, 256)      # k pairs 3-4
    blk(work1, wblkB3, 20, 39, 24, 46, 384, 256)          # q pairs 3-4
    # x2 h0 casts + transpose DMAs, then pull h1 through the stage
    for st in range(4):
        work1.append((15 + st, 29 + st, th_xt_tdma(c, x2T, st, xsA3,
                                                   st)[0]))
    work1.append((19, 33, th_dma(xsA3[:, :, :], x2r[:, 4:8, :])))
    for st in range(4, 8):
        work1.append((25 + st, 40 + st, th_xt_tdma(c, x2T, st, xsA3,
                                                   st - 4)[0]))
    blk(work1, wblkA3, 22, 45, 30, 60, H + 640, 128)      # k pair 5
    blk(work1, wblkB3, 25, 47, 31, 52, 2 * H + 640, 128)  # v heads 10-11
    blk(work1, wblkA3, 31, 61, 34, 62, 640, 128)          # q pair 5
    # v production (each chunk right after its wq16 columns are cast)
    for st in range(ST):
        work1.append((9, 2 * ST, th_v(c, xT, v_x, st, 128, 256,
                                      2, 4)[0]))
        work1.append((10, 4 * ST, th_v(c, xT, v_x, st, 256, 384,
                                       4, 6)[0]))
    for st in range(ST):
        work1.append((20, 6 * ST, th_v(c, xT, v_x, st, 384, 512,
                                       6, 8)[0]))
        work1.append((21, 8 * ST, th_v(c, xT, v_x, st, 512, 640,
                                       8, 10)[0]))
    for st in range(ST):
        work1.append((32, 10 * ST, th_v(c, xT, v_x, st, 640, 768,
                                        10, 12)[0]))
    qk_e = {1: 15, 2: 15, 3: 25, 4: 25, 5: 35}
    for p in range(1, NPAIR):
        for f in th_qk(c, xT, p, HIDDEN, kT_x) + th_qk(c, xT, p, 0, qT_x):
            work1.append((qk_e[p], p * 2 * ST, f))
    for p in range(NPAIR):
        for f in th_qk(c, x2T, p, HIDDEN, kT_x2):
            work1.append((34, END, f))
    for st in range(ST):
        work1.append((36, END, th_v(c, x2T, v_x2, st, 0, 512, 0, 8)[0]))
        work1.append((36, END, th_v(c, x2T, v_x2, st, 512, 768,
                                    8, 12)[0]))
    attention(c, qT_x, kT_x, v_x, oT1, den, work1, first_inline=inline)
    wsp.release()
    xsp.release()

    # post-attn1 allocations (xsp/wsp space reused); qT_x2 reuses the
    # now-dead kT_x buffers (kvxp stays alive through attn2)
    qT_x2 = [kvxp.tile([128, S], F16, name=f"qTx2{i}", tag=f"kTx{i}")
             for i in range(NPAIR)]
    wosp = tc.alloc_tile_pool(name="wosp", bufs=2)
    c.ysb = tc.alloc_tile_pool(name="ysb", bufs=2, side="right")

    def th_wout(ct):
        hold = {}

        def fa():
            wt = wosp.tile([128, HIDDEN], F32, name="wof", tag="wof")
            nc.sync.dma_start(wt[:], wo[ct * 128:(ct + 1) * 128, :])
            hold["w"] = wt

        def fb():
            nc.vector.tensor_copy(c.wout16v[:, ct, :], hold["w"][:])
        return [fa, fb]

    def th_denreset(g):
        def f():
            nc.vector.memset(den[g][:], 1.0)
        return f

    # ---------------- attn3 = attn(q, k2, v2) -------------------------
    oT3 = [otp.tile([128, S], F16, name=f"oTb{i}", tag=f"oTb{i}")
           for i in range(NPAIR)]
    work3 = []
    # normalize attn1 (consumes den group g), then reset it for attn3.
    # group g of the running attention is first written at site 4g*ST's
    # sweep tail, so the reset must be emitted before that.
    for g in range(3):
        dl = [0, ST, 4 * ST][g]
        for qh in range(2):
            ch = norm_group(c, oT1, den, g, qh)
            for f in ch:                        # recip reads den[g]
                work3.append((0, dl, f))
        work3.append((0, dl, th_denreset(g)))   # re-init for this attn
    for ct in range(KT):
        fa, fb = th_wout(ct)
        work3.append((0, 2 * ST + ct, fa))
        work3.append((4 + ct, 2 * ST + ct, fb))
    for p in range(NPAIR):
        for f in th_qk(c, x2T, p, 0, qT_x2):
            work3.append((0, END, f))
    for st in range(ST):
        for f in th_proj(c, oT1, y1, st):
            work3.append((5 * ST, END, f))
    attention(c, qT_x, kT_x2, v_x2, oT3, den, work3)
    wosp.release()
    xtp.release()

    # ---------------- attn2 = attn(q2, k2, v2) ------------------------
    oT2 = [otp.tile([128, S], F16, name=f"oTa{i}", tag=f"oTa{i}")
           for i in range(NPAIR)]
    work2 = []
    for g in range(3):
        dl = [0, ST, 4 * ST][g]
        for qh in range(2):
            ch = norm_group(c, oT3, den, g, qh)
            for f in ch:
                work2.append((0, dl, f))
        work2.append((0, dl, th_denreset(g)))
    for st in range(ST):
        for f in th_proj(c, oT3, y3, st):
            work2.append((0, END, f))
    # attn2 self-normalizes mid-flight: den4 group g is fully collected
    # after pair (2g+1)'s qh1 sweep; group 2's qh0 half is ready one
    # sweep earlier (site 88), halving the serial tail.
    at2 = {3 * ST: norm_group(c, oT2, den, 0, 0),
           4 * ST: norm_group(c, oT2, den, 0, 1),
           7 * ST: norm_group(c, oT2, den, 1, 0),
           8 * ST: norm_group(c, oT2, den, 1, 1),
           11 * ST: norm_group(c, oT2, den, 2, 0)}
    attention(c, qT_x2, kT_x2, v_x2, oT2, den, work2, at_site=at2)
    kvxp.release()
    wqp.release()
    x2tp.release()

    # ---------------- tail: normalize g2 qh1 + proj y2 ----------------
    for f in norm_group(c, oT2, den, 2, 1):
        f()
    cyc = [(c.auxp, "auxA"), (c.auxp, "auxB"),
           (c.ovps, "ovA"), (c.ovps, "ovB")]
    cst = [0]

    def psum_cycle():
        cst[0] = (cst[0] + 1) % 4
        return cyc[cst[0]]

    for st in range(ST):
        for f in th_proj(c, oT2, y2, st, psum_cycle=psum_cycle):
            f()

    c.ysb.release()
    c.bcsp.release()
    c.dnp.release()
    c.exps.release()
    c.xnat.release()
    c.auxp.release()
    c.ovps.release()
    c.sps.release()


def build_bass():
    from contextlib import ExitStack
    nc = bacc.Bacc("TRN2", target_bir_lowering=False, debug=False,
                   num_devices=B)
    x = nc.dram_tensor("x", [S, HIDDEN], F32, kind="ExternalInput").ap()
    x2 = nc.dram_tensor("x2", [S, HIDDEN], F32, kind="ExternalInput").ap()
    wq = nc.dram_tensor("Wqkv", [HIDDEN, 3 * HIDDEN], F32,
                        kind="ExternalInput").ap()
    wo = nc.dram_tensor("Wout", [HIDDEN, HIDDEN], F32,
                        kind="ExternalInput").ap()
    bo = nc.dram_tensor("bout", [HIDDEN], F32, kind="ExternalInput").ap()
    y1 = nc.dram_tensor("y1", [S, HIDDEN], F32, kind="ExternalOutput").ap()
    y2 = nc.dram_tensor("y2", [S, HIDDEN], F32, kind="ExternalOutput").ap()
    y3 = nc.dram_tensor("y3", [S, HIDDEN], F32, kind="ExternalOutput").ap()
    with tile.TileContext(nc) as tc:
        with ExitStack() as ctx:
            build_kernel(ctx, tc, x, x2, wq, wo, bo, y1, y2, y3)
    nc.compile()
    return nc


_NC_CACHE = []


def kernel(x, x2, Wqkv, Wout, bout):
    if not _NC_CACHE:
        _NC_CACHE.append(build_bass())
    nc = _NC_CACHE[0]
    in_maps = [
        {"x": np.ascontiguousarray(x[b]), "x2": np.ascontiguousarray(x2[b]),
         "Wqkv": Wqkv, "Wout": Wout, "bout": bout}
        for b in range(B)
    ]
    res = run_bass_kernel_spmd(nc, in_maps, list(range(B)))
    y1 = np.stack([res.results[b]["y1"] for b in range(B)])
    y2 = np.stack([res.results[b]["y2"] for b in range(B)])
    y3 = np.stack([res.results[b]["y3"] for b in range(B)])
    return (y1, y2, y3)
